# revision 1
# baseline (speedup 1.0000x reference)
"""Pairwise distance screen (CellList) kernel for 8 Trainium2 NeuronCores.

Computes the masked dense [N, N] lower-triangular distance matrix:
  out[i, j] = sqrt(|c_i - c_j|^2)  if  j < i, both species valid, d2 <= cutoff^2
            = 0                    otherwise
with d2 evaluated with exactly the same f32 operation order as the reference
(diff -> square -> sum), so the cutoff mask decisions match bit for bit.

Strategy:
  - Rows are split into 48 blocks of 128. Core c owns blocks
    sorted([c, c+8, c+16, 31-c, 39-c, 47-c]); slot r of every core is padded
    to WMAX[r] = 8*(r+1) col-blocks so all 8 cores share one SPMD program.
    Columns past a core's true diagonal are zeroed by the tril mask; columns
    past the padded width are never written (outputs are donated zero
    buffers).
  - Coordinates are broadcast along partitions bit-exactly by the tensor
    engine: x = xh + xm + xl (exact 3-way bf16 split), K=3 ones-matmul
    accumulated in fp32 PSUM.
  - DVE custom op SQDIFF2 computes (xj-xi)^2 + (yj-yi)^2 in one pass;
    custom op SCREEN_DZ adds dz^2, applies the tril mask (Idx scan vs
    per-partition threshold) and the cutoff compare (t < nextafter(cut2)
    == t <= cut2), and selects t or 0.
  - ACT computes dz = zB - zi (Identity w/ bias) and the final sqrt;
    sqrt(0) = 0 keeps masked entries at zero.
"""

import threading

import numpy as np

N = 6144
P = 128
NCORES = 8
MMW = 512  # matmul free-dim width (one PSUM bank)
SEG = 2048  # y/z broadcast segment width

_lock = threading.Lock()
_cache: dict = {}


def _blocks_for_core(c: int) -> list[int]:
    return sorted([c, c + 8, c + 16, 31 - c, 39 - c, 47 - c])


def _chunk_schedule():
    """(slot, col0, width) pieces; slot r is padded to 1024*(r+1) columns,
    split into 2048-wide pieces plus a trailing 1024 one when odd.
    Ordered so early pieces only need the first broadcast segments and the
    final pieces are small (fast tail flush)."""
    sched = []
    for r in range(6):
        W = 1024 * (r + 1)
        c0 = 0
        while W - c0 >= 2048:
            sched.append((r, c0, 2048))
            c0 += 2048
        if W - c0 > 0:
            sched.append((r, c0, W - c0))
    # Interleave so consumers of later broadcast segments start as late as
    # possible (hides the y/z broadcast DMA latency), and end with the
    # smallest piece for a fast tail flush.
    order = [
        (0, 0, 1024),
        (1, 0, 2048),
        (5, 0, 2048),
        (4, 0, 2048),
        (3, 0, 2048),
        (2, 0, 2048),
        (5, 2048, 2048),
        (4, 2048, 2048),
        (5, 4096, 2048),
        (3, 2048, 2048),
        (2, 2048, 1024),
        (4, 4096, 1024),
    ]
    assert sorted(order) == sorted(sched)
    return order


def _register_ops():
    """Register the two fused DVE ops at runtime (visible to table-gen)."""
    import concourse.dve_ops as dve_ops
    from concourse.dve_spec import (
        C0,
        C1,
        Idx,
        Spec,
        Src0,
        Src1,
        Zero,
        _has_src1,
        lower,
        select,
        sq,
    )
    from concourse.dve_uop import DveOpSpec

    def make(name, body, ref):
        for op in dve_ops.OPS:
            if op.name == name:
                return op
        spec = Spec(body=body, reference=ref)
        row = 1 + len(dve_ops.OPS)
        assert row < 0x20
        shas = {}
        for ver in ("v3", "v4"):
            uops = lower(spec, ver=ver)
            shas[ver] = DveOpSpec(
                name=name, opcode=row, uops=uops, rd1_en=_has_src1(spec)
            ).sha(ver)
        op = dve_ops.DveOp(name, spec, subdim=False, uops_sha=shas)
        dve_ops._SUB_OPCODE_FOR_NAME[name] = row
        dve_ops.OPS.append(op)
        dve_ops.CUSTOM_DVE_SPECS[name] = spec
        return op

    # out = (in0 - s0)^2 + (in1 - s1)^2
    sqdiff2 = make(
        "SQDIFF2_ANT",
        sq(Src0 - C0) + sq(Src1 - C1),
        lambda in0, in1, s0, s1, imm2: (
            (in0.astype(np.float32) - s0) ** 2 + (in1.astype(np.float32) - s1) ** 2
        ).astype(np.float32),
    )

    # t = in0^2 + in1 ; out = (Idx < s0) & (t < s1) ? t : 0
    def screen_ref(in0, in1, s0, s1, imm2):
        t = (in0.astype(np.float32) ** 2 + in1.astype(np.float32)).astype(np.float32)
        idx = np.arange(t.shape[-1], dtype=np.float32)[None, :]
        keep = (idx < s0) & (t < s1)
        return np.where(keep, t, 0.0).astype(np.float32)

    t = sq(Src0) + Src1
    screen = make(
        "SCREEN_DZ_ANT",
        select((Idx < C0) & (t < C1), t, Zero),
        screen_ref,
    )
    return sqdiff2, screen


def _build_program():
    import concourse.bacc as bacc
    import concourse.mybir as mybir
    import concourse.tile as tile

    sqdiff2, screen = _register_ops()

    nc = bacc.Bacc("TRN2", target_bir_lowering=False, debug=False, num_devices=NCORES)
    f32 = mybir.dt.float32
    bf16 = mybir.dt.bfloat16
    Ident = mybir.ActivationFunctionType.Identity

    splits = nc.dram_tensor("splits", [3, N], bf16, kind="ExternalInput")
    ybc = nc.dram_tensor("ybc", [1, N], f32, kind="ExternalInput")
    zbc = nc.dram_tensor("zbc", [1, N], f32, kind="ExternalInput")
    xi6 = nc.dram_tensor("xi6", [P, 6], f32, kind="ExternalInput")
    yi6 = nc.dram_tensor("yi6", [P, 6], f32, kind="ExternalInput")
    nzi6 = nc.dram_tensor("nzi6", [P, 6], f32, kind="ExternalInput")
    cuthi = nc.dram_tensor("cuthi", [P, 1], f32, kind="ExternalInput")
    idxthr = nc.dram_tensor("idxthr", [P, 12], f32, kind="ExternalInput")
    out = nc.dram_tensor("out", [6 * P, N], f32, kind="ExternalOutput")

    sched = _chunk_schedule()
    nseg = N // SEG  # 3

    with tile.TileContext(nc) as tc:
        with (
            tc.tile_pool(name="const", bufs=1) as cpool,
            tc.tile_pool(name="work", bufs=3) as wpool,
            tc.tile_pool(name="dzp", bufs=4) as dzpool,
            tc.tile_pool(name="outp", bufs=4) as spool,
            tc.tile_pool(name="psx", bufs=2, space="PSUM") as ppx,
        ):
            splits_t = cpool.tile([3, N], bf16, tag="splits")
            ones_t = cpool.tile([3, P], bf16, tag="ones")
            xi_t = cpool.tile([P, 6], f32, tag="xi")
            yi_t = cpool.tile([P, 6], f32, tag="yi")
            nzi_t = cpool.tile([P, 6], f32, tag="nzi")
            cut_t = cpool.tile([P, 1], f32, tag="cut")
            ithr_t = cpool.tile([P, 12], f32, tag="ithr")
            yB = [
                cpool.tile([P, SEG], f32, tag=f"yB{m}", name=f"yB{m}")
                for m in range(nseg)
            ]
            zB = [
                cpool.tile([P, SEG], f32, tag=f"zB{m}", name=f"zB{m}")
                for m in range(nseg)
            ]
            warm_t = cpool.tile([P, 2], f32, tag="warm")

            # pull the ACT function tables in immediately (no DMA deps)
            nc.vector.memset(warm_t[:, 0:1], 1.0)
            nc.scalar.activation(
                warm_t[:, 1:2], warm_t[:, 0:1], Ident, bias=0.0, scale=1.0
            )
            nc.scalar.sqrt(warm_t[:, 0:1], warm_t[:, 1:2])

            # spread input DMA issue across sequencers; first-needed first
            nc.sync.dma_start(splits_t[:], splits[:])
            sg = slice(0, SEG)
            nc.gpsimd.dma_start(zB[0][:], zbc[0:1, sg].partition_broadcast(P))
            nc.sync.dma_start(yB[0][:], ybc[0:1, sg].partition_broadcast(P))
            nc.gpsimd.dma_start(nzi_t[:], nzi6[:])
            nc.gpsimd.dma_start(xi_t[:], xi6[:])
            nc.gpsimd.dma_start(yi_t[:], yi6[:])
            nc.gpsimd.dma_start(cut_t[:], cuthi[:])
            nc.gpsimd.dma_start(ithr_t[:], idxthr[:])
            nc.gpsimd.memset(ones_t[:], 1.0)
            for m in range(1, nseg):
                sg = slice(m * SEG, (m + 1) * SEG)
                nc.gpsimd.dma_start(zB[m][:], zbc[0:1, sg].partition_broadcast(P))
                nc.sync.dma_start(yB[m][:], ybc[0:1, sg].partition_broadcast(P))
            # pull the ACT function tables in during startup
            nc.scalar.activation(warm_t[:, 0:1], cut_t[:], Ident, bias=0.0, scale=1.0)
            nc.scalar.sqrt(warm_t[:, 1:2], cut_t[:])

            # dz ops are emitted a few pieces ahead of their consumers so the
            # in-order scalar queue never parks a ready dz behind a blocked
            # sqrt (convoy stall on DVE).
            dzs = {}

            def emit_dz(idx):
                if idx >= len(sched):
                    return
                r, c0, w = sched[idx]
                m, off = divmod(c0, SEG)
                dz = dzpool.tile([P, w], f32, tag="dz", name=f"dz{idx}")
                nc.scalar.activation(
                    dz[:],
                    zB[m][:, off : off + w],
                    Ident,
                    bias=nzi_t[:, r : r + 1],
                    scale=1.0,
                )
                dzs[idx] = dz

            for i in range(3):
                emit_dz(i)

            for cnt, (r, c0, w) in enumerate(sched):
                emit_dz(cnt + 3)
                xb = ppx.tile([P, w], f32, tag="xb")
                for h in range(0, w, MMW):
                    nc.tensor.matmul(
                        xb[:, h : h + MMW],
                        ones_t[:],
                        splits_t[:, c0 + h : c0 + h + MMW],
                        start=True,
                        stop=True,
                    )
                dxy2 = wpool.tile([P, w], f32, tag="dxy2")
                nc.vector._custom_dve(
                    sqdiff2,
                    out=dxy2[:],
                    in0=xb[:],
                    in1=yB[c0 // SEG][:, c0 % SEG : c0 % SEG + w],
                    s0=xi_t[:, r : r + 1],
                    s1=yi_t[:, r : r + 1],
                )
                v = wpool.tile([P, w], f32, tag="v")
                nc.vector._custom_dve(
                    screen,
                    out=v[:],
                    in0=dzs.pop(cnt)[:],
                    in1=dxy2[:],
                    s0=ithr_t[:, cnt : cnt + 1],
                    s1=cut_t[:],
                )
                s = spool.tile([P, w], f32, tag="s")
                nc.scalar.sqrt(s[:], v[:])
                nc.sync.dma_start(out[r * P : (r + 1) * P, c0 : c0 + w], s[:])

    nc.compile()
    return nc


def _get_program():
    with _lock:
        if "nc" not in _cache:
            _cache["nc"] = _build_program()
    return _cache["nc"]


def _split3_bf16(v32: np.ndarray):
    """Exact 3-way bf16 split: v32 == hi + mid + lo (as f32 sums, any order)."""
    import ml_dtypes

    bf = ml_dtypes.bfloat16
    hi = v32.astype(bf)
    r1 = (v32 - hi.astype(np.float32)).astype(np.float32)
    mid = r1.astype(bf)
    lo = (r1 - mid.astype(np.float32)).astype(np.float32).astype(bf)
    # verify exactness (cheap); required for the bit-exact mask
    recon = (
        hi.astype(np.float32) + mid.astype(np.float32) + lo.astype(np.float32)
    ).astype(np.float32)
    assert np.array_equal(recon, v32), "bf16 3-way split not exact"
    return hi, mid, lo


def _prepare_inputs(species, coordinates, cutoff):
    coords = np.asarray(coordinates, dtype=np.float32).reshape(-1, 3).copy()
    assert coords.shape[0] == N
    valid = np.asarray(species).reshape(-1) >= 0
    if not valid.all():
        bad = np.where(~valid)[0]
        coords[bad] = (1.0e5 + 1.0e4 * np.arange(len(bad), dtype=np.float32))[:, None]

    x, y, z = coords[:, 0].copy(), coords[:, 1].copy(), coords[:, 2].copy()

    import ml_dtypes

    hi, mid, lo = _split3_bf16(x)
    splits = np.stack(
        [hi.astype(np.float32), mid.astype(np.float32), lo.astype(np.float32)]
    ).astype(ml_dtypes.bfloat16)
    ybc = np.ascontiguousarray(y[None, :])
    zbc = np.ascontiguousarray(z[None, :])

    cut2 = np.float32(cutoff) * np.float32(cutoff)
    cut_hi = np.nextafter(cut2, np.float32(np.inf), dtype=np.float32)
    cuthi = np.full((P, 1), cut_hi, np.float32)

    sched = _chunk_schedule()
    in_maps = []
    for c in range(NCORES):
        blocks = _blocks_for_core(c)
        rows = np.concatenate([np.arange(P * b, P * b + P) for b in blocks])
        rmat = rows.reshape(6, P)  # [slot, partition]
        xi6 = np.ascontiguousarray(x[rmat].T)  # [128, 6]
        yi6 = np.ascontiguousarray(y[rmat].T)
        nzi6 = np.ascontiguousarray(-z[rmat].T)
        idxthr = np.empty((P, len(sched)), np.float32)
        for cnt, (r, c0, w) in enumerate(sched):
            idxthr[:, cnt] = rmat[r].astype(np.float32) - np.float32(c0)
        in_maps.append(
            {
                "splits": splits,
                "ybc": ybc,
                "zbc": zbc,
                "xi6": xi6,
                "yi6": yi6,
                "nzi6": nzi6,
                "cuthi": cuthi,
                "idxthr": idxthr,
            }
        )
    return in_maps


def _run(in_maps, trace=False):
    from concourse import bass_utils

    nc = _get_program()
    return bass_utils.run_bass_kernel_spmd(
        nc, in_maps, core_ids=list(range(NCORES)), trace=trace
    )


def _assemble(results):
    full = np.zeros((N, N), np.float32)
    for c in range(NCORES):
        o = results[c]["out"]
        for r, b in enumerate(_blocks_for_core(c)):
            full[P * b : P * (b + 1), :] = o[P * r : P * (r + 1), :]
    return full


def kernel(species, coordinates, cutoff):
    in_maps = _prepare_inputs(species, coordinates, cutoff)
    res = _run(in_maps)
    return _assemble(res.results)



# revision 2
# speedup vs baseline: 1.8208x; 1.8208x over previous
"""Pairwise distance screen (CellList) kernel for 8 Trainium2 NeuronCores.

Computes the masked dense [N, N] lower-triangular distance matrix:
  out[i, j] = sqrt(|c_i - c_j|^2)  if  j < i, both species valid, d2 <= cutoff^2
            = 0                    otherwise
with d2 evaluated with exactly the same f32 operation order as the reference
(diff -> square -> sum), so the cutoff mask decisions match bit for bit.

Strategy (spatial banding):
  - Atoms are sorted by x on the host. Any pair within the cutoff has
    |x_i - x_j| <= cutoff, so in sorted order row-block b (rows
    [128b, 128b+128)) only interacts with a ~W-wide contiguous column
    window ending at its own diagonal: cols [128(b+1)-W, 128(b+1)).
    W = 1024 covers the worst block with margin (verified at runtime from
    the data; the program is rebuilt wider if ever insufficient).
  - Core c owns 6 consecutive blocks 6c..6c+5. Their windows overlap, so
    the core only needs a UNION = W + 5*128 column slice of the sorted
    arrays. All 8 cores run one SPMD program; the per-core window position
    lives entirely in the packed input data (and the host-side unpack).
  - x_j is broadcast along partitions bit-exactly by the tensor engine
    (3-way bf16 split, ones-matmul into fp32 PSUM). y_j / z_j are
    partition-broadcast by DMA (exact f32 copies).
  - DVE op SQDIFF2 computes (xj-xi)^2 + (yj-yi)^2 in one pass; DVE op
    SCREEN3 computes t = (zj-zi)^2 + dxy2, then selects t or 0 by
    t < nextafter(cut2) (== t <= cut2).  ACT computes sqrt -> bf16.
  - The host scatters the nonzero entries of each slab to
    full[max(oi,oj), min(oi,oj)] through the sort permutation. Pairs in a
    diagonal block appear twice with bit-identical values, so duplicate
    scatter writes are benign.  Screened / out-of-window / sentinel
    entries are exact zeros and are never scattered.
"""

import threading

import numpy as np

N = 6144
P = 128
NCORES = 8
BPC = 6  # row-blocks per core (consecutive)
NB = N // P  # 48
W0 = 1024  # default slot window width (multiple of 128)
MMW = 512  # matmul free-dim width (one PSUM bank)

_lock = threading.Lock()
_cache: dict = {}


def _register_ops():
    """Register the two fused DVE ops at runtime (visible to table-gen)."""
    import concourse.dve_ops as dve_ops
    from concourse.dve_spec import (
        C0,
        C1,
        Spec,
        Src0,
        Src1,
        Zero,
        _has_src1,
        lower,
        select,
        sq,
    )
    from concourse.dve_uop import DveOpSpec

    def make(name, body, ref):
        for op in dve_ops.OPS:
            if op.name == name:
                return op
        spec = Spec(body=body, reference=ref)
        row = 1 + len(dve_ops.OPS)
        assert row < 0x20
        shas = {}
        for ver in ("v3", "v4"):
            uops = lower(spec, ver=ver)
            shas[ver] = DveOpSpec(
                name=name, opcode=row, uops=uops, rd1_en=_has_src1(spec)
            ).sha(ver)
        op = dve_ops.DveOp(name, spec, subdim=False, uops_sha=shas)
        dve_ops._SUB_OPCODE_FOR_NAME[name] = row
        dve_ops.OPS.append(op)
        dve_ops.CUSTOM_DVE_SPECS[name] = spec
        return op

    # out = (in0 - s0)^2 + (in1 - s1)^2
    sqdiff2 = make(
        "SQDIFF2_ANT",
        sq(Src0 - C0) + sq(Src1 - C1),
        lambda in0, in1, s0, s1, imm2: (
            (in0.astype(np.float32) - s0) ** 2 + (in1.astype(np.float32) - s1) ** 2
        ).astype(np.float32),
    )

    # t = (in0 - s0)^2 + in1 ; out = (t < s1) ? t : 0
    def screen_ref(in0, in1, s0, s1, imm2):
        t = ((in0.astype(np.float32) - s0) ** 2 + in1.astype(np.float32)).astype(
            np.float32
        )
        return np.where(t < s1, t, 0.0).astype(np.float32)

    t = sq(Src0 - C0) + Src1
    screen = make(
        "SCREEN3_ANT",
        select(t < C1, t, Zero),
        screen_ref,
    )
    return sqdiff2, screen


def _build_program(W):
    import concourse.bacc as bacc
    import concourse.mybir as mybir
    import concourse.tile as tile

    sqdiff2, screen = _register_ops()

    UNION = W + (BPC - 1) * P

    nc = bacc.Bacc("TRN2", target_bir_lowering=False, debug=False, num_devices=NCORES)
    f32 = mybir.dt.float32
    bf16 = mybir.dt.bfloat16

    splits = nc.dram_tensor("splits", [3, UNION], bf16, kind="ExternalInput")
    ybc = nc.dram_tensor("ybc", [1, UNION], f32, kind="ExternalInput")
    zbc = nc.dram_tensor("zbc", [1, UNION], f32, kind="ExternalInput")
    xi6 = nc.dram_tensor("xi6", [P, BPC], f32, kind="ExternalInput")
    yi6 = nc.dram_tensor("yi6", [P, BPC], f32, kind="ExternalInput")
    zi6 = nc.dram_tensor("zi6", [P, BPC], f32, kind="ExternalInput")
    cuthi = nc.dram_tensor("cuthi", [P, 1], f32, kind="ExternalInput")
    out = nc.dram_tensor("out", [BPC * P, W], bf16, kind="ExternalOutput")

    # broadcast chunk split: first chunk covers slot 0's window, second the rest
    CH0 = W
    with tile.TileContext(nc) as tc:
        with (
            tc.tile_pool(name="const", bufs=1) as cpool,
            tc.tile_pool(name="work", bufs=3) as wpool,
            tc.tile_pool(name="outp", bufs=3) as spool,
            tc.tile_pool(name="psx", bufs=1, space="PSUM") as ppx,
        ):
            splits_t = cpool.tile([3, UNION], bf16, tag="splits")
            ones_t = cpool.tile([3, P], bf16, tag="ones")
            xi_t = cpool.tile([P, BPC], f32, tag="xi")
            yi_t = cpool.tile([P, BPC], f32, tag="yi")
            zi_t = cpool.tile([P, BPC], f32, tag="zi")
            cut_t = cpool.tile([P, 1], f32, tag="cut")
            yB = cpool.tile([P, UNION], f32, tag="yB")
            zB = cpool.tile([P, UNION], f32, tag="zB")
            warm_t = cpool.tile([P, 2], f32, tag="warm")

            # pull the ACT sqrt table in immediately (no DMA deps)
            nc.vector.memset(warm_t[:, 0:1], 1.0)
            nc.scalar.sqrt(warm_t[:, 1:2], warm_t[:, 0:1])

            # first-needed-first DMA issue, spread across sequencers
            nc.sync.dma_start(yB[:, 0:CH0], ybc[0:1, 0:CH0].partition_broadcast(P))
            nc.gpsimd.dma_start(zB[:, 0:CH0], zbc[0:1, 0:CH0].partition_broadcast(P))
            nc.scalar.dma_start(splits_t[:], splits[:])
            nc.gpsimd.dma_start(xi_t[:], xi6[:])
            nc.gpsimd.dma_start(yi_t[:], yi6[:])
            nc.gpsimd.dma_start(zi_t[:], zi6[:])
            nc.gpsimd.dma_start(cut_t[:], cuthi[:])
            nc.gpsimd.memset(ones_t[:], 1.0)
            nc.sync.dma_start(
                yB[:, CH0:UNION], ybc[0:1, CH0:UNION].partition_broadcast(P)
            )
            nc.gpsimd.dma_start(
                zB[:, CH0:UNION], zbc[0:1, CH0:UNION].partition_broadcast(P)
            )

            xb = ppx.tile([P, 2048], f32, tag="xb")
            for h in range(0, UNION, MMW):
                hw = min(MMW, UNION - h)
                nc.tensor.matmul(
                    xb[:, h : h + hw],
                    ones_t[:],
                    splits_t[:, h : h + hw],
                    start=True,
                    stop=True,
                )

            for k in range(BPC):
                o = k * P
                dxy2 = wpool.tile([P, W], f32, tag="dxy2")
                nc.vector._custom_dve(
                    sqdiff2,
                    out=dxy2[:],
                    in0=xb[:, o : o + W],
                    in1=yB[:, o : o + W],
                    s0=xi_t[:, k : k + 1],
                    s1=yi_t[:, k : k + 1],
                )
                v = wpool.tile([P, W], f32, tag="v")
                nc.vector._custom_dve(
                    screen,
                    out=v[:],
                    in0=zB[:, o : o + W],
                    in1=dxy2[:],
                    s0=zi_t[:, k : k + 1],
                    s1=cut_t[:],
                )
                s = spool.tile([P, W], bf16, tag="s")
                nc.scalar.sqrt(s[:], v[:])
                nc.sync.dma_start(out[k * P : (k + 1) * P, :], s[:])

    nc.compile()
    return nc


def _get_program(W):
    with _lock:
        key = ("nc", W)
        if key not in _cache:
            _cache[key] = _build_program(W)
    return _cache[key]


def _split3_bf16(v32: np.ndarray):
    """Exact 3-way bf16 split: v32 == (hi + mid) + lo in f32."""
    import ml_dtypes

    bf = ml_dtypes.bfloat16
    hi = v32.astype(bf)
    r1 = (v32 - hi.astype(np.float32)).astype(np.float32)
    mid = r1.astype(bf)
    lo = (r1 - mid.astype(np.float32)).astype(np.float32).astype(bf)
    recon = (
        hi.astype(np.float32) + mid.astype(np.float32) + lo.astype(np.float32)
    ).astype(np.float32)
    assert np.array_equal(recon, v32), "bf16 3-way split not exact"
    return hi, mid, lo


def _prepare_inputs(species, coordinates, cutoff):
    import ml_dtypes

    coords = np.asarray(coordinates, dtype=np.float32).reshape(-1, 3).copy()
    assert coords.shape[0] == N
    valid = np.asarray(species).reshape(-1) >= 0
    if not valid.all():
        bad = np.where(~valid)[0]
        coords[bad] = (1.0e5 + 1.0e4 * np.arange(len(bad), dtype=np.float32))[:, None]

    cutf = np.float32(cutoff)
    perm = np.argsort(coords[:, 0], kind="stable")
    sx = np.ascontiguousarray(coords[perm, 0])
    sy = np.ascontiguousarray(coords[perm, 1])
    sz = np.ascontiguousarray(coords[perm, 2])

    # minimum window width so every block's cutoff neighborhood is covered
    bmins = sx[:: P][:NB]
    j0 = np.searchsorted(sx, bmins - cutf, side="left")
    need = int((P * (np.arange(NB) + 1) - j0).max())
    W = max(W0, -(-need // P) * P)
    UNION = W + (BPC - 1) * P
    PADL = UNION - BPC * P

    gx = np.concatenate([np.full(PADL, -1.0e4, np.float32), sx])
    gy = np.concatenate([np.zeros(PADL, np.float32), sy])
    gz = np.concatenate([np.zeros(PADL, np.float32), sz])

    hi, mid, lo = _split3_bf16(gx)
    gsplits = np.stack(
        [hi.astype(np.float32), mid.astype(np.float32), lo.astype(np.float32)]
    ).astype(ml_dtypes.bfloat16)

    cut2 = cutf * cutf
    cut_hi = np.nextafter(cut2, np.float32(np.inf), dtype=np.float32)
    cuthi = np.full((P, 1), cut_hi, np.float32)

    in_maps = []
    for c in range(NCORES):
        base = PADL + P * BPC * (c + 1) - UNION
        rows = np.arange(P * BPC * c, P * BPC * (c + 1)).reshape(BPC, P)
        in_maps.append(
            {
                "splits": np.ascontiguousarray(gsplits[:, base : base + UNION]),
                "ybc": np.ascontiguousarray(gy[None, base : base + UNION]),
                "zbc": np.ascontiguousarray(gz[None, base : base + UNION]),
                "xi6": np.ascontiguousarray(sx[rows].T),
                "yi6": np.ascontiguousarray(sy[rows].T),
                "zi6": np.ascontiguousarray(sz[rows].T),
                "cuthi": cuthi,
            }
        )
    _cache["meta"] = (perm, W)
    return in_maps


def _run(in_maps, trace=False):
    from concourse import bass_utils

    nc = _get_program(_cache["meta"][1])
    return bass_utils.run_bass_kernel_spmd(
        nc, in_maps, core_ids=list(range(NCORES)), trace=trace
    )


def _assemble(results, perm, W):
    full = np.zeros((N, N), np.float32)
    for c in range(NCORES):
        slab = np.asarray(results[c]["out"]).astype(np.float32)
        for k in range(BPC):
            b = BPC * c + k
            start = P * (b + 1) - W
            t0 = max(0, -start)
            vals = slab[k * P : (k + 1) * P, t0:]
            rr, cc = np.nonzero(vals)
            if rr.size == 0:
                continue
            oi = perm[P * b + rr]
            oj = perm[start + t0 + cc]
            hi = np.maximum(oi, oj)
            lo = np.minimum(oi, oj)
            full[hi, lo] = vals[rr, cc]
    return full


def kernel(species, coordinates, cutoff):
    in_maps = _prepare_inputs(species, coordinates, cutoff)
    res = _run(in_maps)
    perm, W = _cache["meta"]
    return _assemble(res.results, perm, W)


# revision 3
# speedup vs baseline: 2.9178x; 1.6025x over previous
"""Pairwise distance screen (CellList) kernel for 8 Trainium2 NeuronCores.

Computes the masked dense [N, N] lower-triangular distance matrix:
  out[i, j] = sqrt(|c_i - c_j|^2)  if  j < i, both species valid, d2 <= cutoff^2
            = 0                    otherwise

Strategy (spatial banding + bilinear distance on the tensor engine):
  - Atoms are sorted by x on the host. Any pair within the cutoff has
    |x_i - x_j| <= cutoff, so in sorted order row-block b (rows
    [128b, 128b+128)) only interacts with a ~W-wide contiguous column
    window ending at its own diagonal. W = 1024 covers the worst block
    with margin (verified at runtime; rebuilt wider if insufficient).
  - Core c owns 6 consecutive blocks 6c..6c+5; it only needs a
    UNION = W + 5*128 column slice. One SPMD program; all per-core
    window placement lives in the packed input data + host unpack.
  - d2 is computed bilinearly on the PE: d2 = r2_j - 2*ci.cj + r2_i,
    with coordinates centered per core and 3-way bf16 split so every
    product is exact; the K=21 matmul accumulates
    -2*ci.cj + r2_j in fp32 PSUM (6 product terms per dim keep all
    cross terms above ~2^-24; r2_j is a 3-way split of the f64 value).
    Accumulated |d2 error| ~1e-4, so only O(1) pairs near the cutoff
    boundary can flip vs the reference mask (~5e-3 Frobenius budget).
  - DVE op ADDSEL adds r2_i ([P,1] per-partition) and selects
    relu(t) if t < nextafter(cut2) else 0 (relu guards sqrt(-eps) on
    the diagonal).  ACT computes sqrt -> bf16.
  - The host scatters nonzero entries of each slab to
    full[max(oi,oj), min(oi,oj)] through the sort permutation,
    dropping sentinel-padding columns and dummy (species<0) atoms.
    Diagonal-block pairs appear twice with near-identical values;
    duplicate scatter writes are benign.
"""

import threading

import numpy as np

N = 6144
P = 128
NCORES = 8
BPC = 6  # row-blocks per core (consecutive)
NB = N // P  # 48
W0 = 1024  # default slot window width (multiple of 128)
MMW = 512  # matmul free-dim width (one PSUM bank)
K = 21  # 6 product terms per dim + 3 r2 rows

_lock = threading.Lock()
_cache: dict = {}


def _register_ops():
    """Register the fused DVE op at runtime (visible to table-gen)."""
    import concourse.dve_ops as dve_ops
    from concourse.dve_spec import (
        C0,
        C1,
        Spec,
        Src0,
        Zero,
        _has_src1,
        lower,
        relu,
        select,
    )
    from concourse.dve_uop import DveOpSpec

    def make(name, body, ref):
        for op in dve_ops.OPS:
            if op.name == name:
                return op
        spec = Spec(body=body, reference=ref)
        row = 1 + len(dve_ops.OPS)
        assert row < 0x20
        shas = {}
        for ver in ("v3", "v4"):
            uops = lower(spec, ver=ver)
            shas[ver] = DveOpSpec(
                name=name, opcode=row, uops=uops, rd1_en=_has_src1(spec)
            ).sha(ver)
        op = dve_ops.DveOp(name, spec, subdim=False, uops_sha=shas)
        dve_ops._SUB_OPCODE_FOR_NAME[name] = row
        dve_ops.OPS.append(op)
        dve_ops.CUSTOM_DVE_SPECS[name] = spec
        return op

    # t = in0 + s0 ; out = (t < s1) ? max(t, 0) : 0
    def addsel_ref(in0, in1, s0, s1, imm2):
        t = (in0.astype(np.float32) + s0).astype(np.float32)
        return np.where(t < s1, np.maximum(t, 0.0), 0.0).astype(np.float32)

    t = Src0 + C0
    addsel = make("ADDSEL_ANT", select(t < C1, relu(t), Zero), addsel_ref)
    return addsel


def _build_program(W):
    import concourse.bacc as bacc
    import concourse.mybir as mybir
    import concourse.tile as tile

    addsel = _register_ops()

    UNION = W + (BPC - 1) * P
    WCOLS = BPC * P  # 768 weight columns

    nc = bacc.Bacc("TRN2", target_bir_lowering=False, debug=False, num_devices=NCORES)
    f32 = mybir.dt.float32
    bf16 = mybir.dt.bfloat16

    # weights [:, :WCOLS] ++ moving [:, WCOLS:]
    wtmov = nc.dram_tensor("wtmov", [K, WCOLS + UNION], bf16, kind="ExternalInput")
    consts = nc.dram_tensor("consts", [P, 8], f32, kind="ExternalInput")
    out = nc.dram_tensor("out", [BPC * P, W], bf16, kind="ExternalOutput")

    with tile.TileContext(nc) as tc:
        with (
            tc.tile_pool(name="const", bufs=1) as cpool,
            tc.tile_pool(name="work", bufs=3) as wpool,
            tc.tile_pool(name="outp", bufs=3) as spool,
            tc.tile_pool(name="psx", bufs=3, space="PSUM") as ppx,
        ):
            wm_t = cpool.tile([K, WCOLS + UNION], bf16, tag="wtmov")
            c_t = cpool.tile([P, 8], f32, tag="consts")
            warm_t = cpool.tile([P, 2], f32, tag="warm")

            # pull the ACT sqrt table in immediately (no DMA deps)
            nc.vector.memset(warm_t[:, 0:1], 1.0)
            nc.scalar.sqrt(warm_t[:, 1:2], warm_t[:, 0:1])

            nc.sync.dma_start(wm_t[:], wtmov[:])
            nc.gpsimd.dma_start(c_t[:], consts[:])

            for k in range(BPC):
                o = WCOLS + k * P
                xb = ppx.tile([P, W], f32, tag="xb")
                for h in range(0, W, MMW):
                    nc.tensor.matmul(
                        xb[:, h : h + MMW],
                        wm_t[:, k * P : (k + 1) * P],
                        wm_t[:, o + h : o + h + MMW],
                        start=True,
                        stop=True,
                    )
                v = wpool.tile([P, W], f32, tag="v")
                nc.vector._custom_dve(
                    addsel,
                    out=v[:],
                    in0=xb[:],
                    s0=c_t[:, k : k + 1],
                    s1=c_t[:, 6:7],
                )
                s = spool.tile([P, W], bf16, tag="s")
                nc.scalar.sqrt(s[:], v[:])
                nc.sync.dma_start(out[k * P : (k + 1) * P, :], s[:])

    nc.compile()
    return nc


def _get_program(W):
    with _lock:
        key = ("nc", W)
        if key not in _cache:
            _cache[key] = _build_program(W)
    return _cache[key]


def _split3_bf16_f64(v64: np.ndarray):
    """3-way bf16 split of float64 values: h+m+l captures ~24 mantissa bits."""
    import ml_dtypes

    bf = ml_dtypes.bfloat16
    h = v64.astype(bf)
    r1 = v64 - h.astype(np.float64)
    m = r1.astype(bf)
    r2 = r1 - m.astype(np.float64)
    l = r2.astype(bf)
    return h, m, l


def _prepare_inputs(species, coordinates, cutoff):
    import ml_dtypes

    bf = ml_dtypes.bfloat16
    coords = np.asarray(coordinates, dtype=np.float32).reshape(-1, 3).copy()
    assert coords.shape[0] == N
    valid = np.asarray(species).reshape(-1) >= 0
    if not valid.all():
        bad = np.where(~valid)[0]
        coords[bad] = (1.0e5 + 1.0e4 * np.arange(len(bad), dtype=np.float32))[:, None]

    cutf = np.float32(cutoff)
    perm = np.argsort(coords[:, 0], kind="stable")
    sx = np.ascontiguousarray(coords[perm, 0])
    sy = np.ascontiguousarray(coords[perm, 1])
    sz = np.ascontiguousarray(coords[perm, 2])
    svalid = valid[perm]

    # minimum window width so every block's cutoff neighborhood is covered
    bmins = sx[::P][:NB]
    j0 = np.searchsorted(sx, bmins - cutf, side="left")
    need = int((P * (np.arange(NB) + 1) - j0).max())
    W = max(W0, -(-need // P) * P)
    UNION = W + (BPC - 1) * P
    PADL = UNION - BPC * P
    WCOLS = BPC * P

    gx = np.concatenate([np.full(PADL, -1.0e4, np.float64), sx.astype(np.float64)])
    gy = np.concatenate([np.zeros(PADL, np.float64), sy.astype(np.float64)])
    gz = np.concatenate([np.zeros(PADL, np.float64), sz.astype(np.float64)])
    gvalid = np.concatenate([np.zeros(PADL, bool), svalid])

    cut2 = cutf * cutf
    cut_hi = np.nextafter(cut2, np.float32(np.inf), dtype=np.float32)

    def rows21(xc, yc, zc, r2):
        rows = []
        for d in (xc, yc, zc):
            h, m, l = _split3_bf16_f64(d)
            rows += [h, m, l, h, m, h]
        ra, rb, rc = _split3_bf16_f64(r2)
        rows += [ra, rb, rc]
        return rows

    in_maps = []
    for c in range(NCORES):
        base = PADL + WCOLS * (c + 1) - UNION
        rowi = np.arange(WCOLS * c, WCOLS * (c + 1))
        rx = sx[rowi].astype(np.float64)
        ry = sy[rowi].astype(np.float64)
        rz = sz[rowi].astype(np.float64)
        realr = np.abs(rx) <= 1000.0
        cx = float(np.median(rx[realr])) if realr.any() else 0.0
        cy = cz = 20.0

        # moving columns (union slice), centered
        ux = gx[base : base + UNION] - cx
        uy = gy[base : base + UNION] - cy
        uz = gz[base : base + UNION] - cz
        ur2 = ux * ux + uy * uy + uz * uz
        mov = rows21(ux, uy, uz, ur2)

        # stationary per-row weights
        wx, wy, wz = rx - cx, ry - cy, rz - cz
        wr2 = wx * wx + wy * wy + wz * wz
        wrows = []
        for d in (wx, wy, wz):
            h, m, l = _split3_bf16_f64(d)
            n2 = np.float32(-2.0)
            wrows += [
                (n2 * h.astype(np.float32)).astype(bf),
                (n2 * h.astype(np.float32)).astype(bf),
                (n2 * h.astype(np.float32)).astype(bf),
                (n2 * m.astype(np.float32)).astype(bf),
                (n2 * m.astype(np.float32)).astype(bf),
                (n2 * l.astype(np.float32)).astype(bf),
            ]
        ones = np.ones(WCOLS, bf)
        wrows += [ones, ones, ones]

        wtmov = np.empty((K, WCOLS + UNION), bf)
        for r in range(K):
            wtmov[r, :WCOLS] = wrows[r]
            wtmov[r, WCOLS:] = mov[r]

        consts = np.zeros((P, 8), np.float32)
        consts[:, 0:BPC] = wr2.astype(np.float32).reshape(BPC, P).T
        consts[:, 6] = cut_hi
        in_maps.append({"wtmov": wtmov, "consts": consts})

    _cache["meta"] = (perm, W, gvalid, PADL)
    return in_maps


def _run(in_maps, trace=False):
    from concourse import bass_utils

    nc = _get_program(_cache["meta"][1])
    return bass_utils.run_bass_kernel_spmd(
        nc, in_maps, core_ids=list(range(NCORES)), trace=trace
    )


def _assemble(results, perm, W, gvalid, PADL):
    full = np.zeros((N, N), np.float32)
    svalid = gvalid[PADL:]
    for c in range(NCORES):
        slab = np.asarray(results[c]["out"]).astype(np.float32)
        for k in range(BPC):
            b = BPC * c + k
            start = P * (b + 1) - W
            t0 = max(0, -start)
            vals = slab[k * P : (k + 1) * P, t0:]
            rr, cc = np.nonzero(vals)
            if rr.size == 0:
                continue
            si = P * b + rr
            sj = start + t0 + cc
            keep = svalid[si] & svalid[sj]
            if not keep.all():
                rr, cc, si, sj = rr[keep], cc[keep], si[keep], sj[keep]
            oi = perm[si]
            oj = perm[sj]
            hi = np.maximum(oi, oj)
            lo = np.minimum(oi, oj)
            full[hi, lo] = vals[rr, cc]
    return full


def kernel(species, coordinates, cutoff):
    in_maps = _prepare_inputs(species, coordinates, cutoff)
    res = _run(in_maps)
    perm, W, gvalid, PADL = _cache["meta"]
    return _assemble(res.results, perm, W, gvalid, PADL)


# revision 11
# speedup vs baseline: 2.9711x; 1.0183x over previous
"""Pairwise distance screen (CellList) kernel for 8 Trainium2 NeuronCores.

Computes the masked dense [N, N] lower-triangular distance matrix:
  out[i, j] = sqrt(|c_i - c_j|^2)  if  j < i, both species valid, d2 <= cutoff^2
            = 0                    otherwise

Strategy (spatial banding + bilinear distance on the tensor engine):
  - Atoms are sorted by x on the host. Any pair within the cutoff has
    |x_i - x_j| <= cutoff, so in sorted order row-block b (rows
    [128b, 128b+128)) only interacts with a ~W-wide contiguous column
    window ending at its own diagonal. W = 1024 covers the worst block
    with margin (verified at runtime; rebuilt wider if insufficient).
  - Core c owns 6 consecutive blocks 6c..6c+5; it only needs a
    UNION = W + 5*128 column slice. One SPMD program; all per-core
    window placement lives in the packed input data + host unpack.
  - d2 is computed bilinearly on the PE: d2 = r2_j - 2*ci.cj + r2_i,
    with coordinates centered per core and 3-way bf16 split so every
    product is exact; the K=21 matmul accumulates
    -2*ci.cj + r2_j in fp32 PSUM (6 product terms per dim keep all
    cross terms above ~2^-24; r2_j is a 3-way split of the f64 value).
    Accumulated |d2 error| ~1e-4, so only O(1) pairs near the cutoff
    boundary can flip vs the reference mask (~5e-3 Frobenius budget).
  - DVE op ADDSEL adds r2_i ([P,1] per-partition) and selects
    relu(t) if t < nextafter(cut2) else 0 (relu guards sqrt(-eps) on
    the diagonal).  ACT computes sqrt -> bf16.
  - The host scatters nonzero entries of each slab to
    full[max(oi,oj), min(oi,oj)] through the sort permutation,
    dropping sentinel-padding columns and dummy (species<0) atoms.
    Diagonal-block pairs appear twice with near-identical values;
    duplicate scatter writes are benign.
"""

import threading

import numpy as np

N = 6144
P = 128
NCORES = 8
BPC = 6  # row-blocks per core (consecutive)
NB = N // P  # 48
W0 = 1024  # default slot window width (multiple of 128)
MMW = 512  # matmul free-dim width (one PSUM bank)
K = 21  # 6 product terms per dim + 3 r2 rows

_lock = threading.Lock()
_cache: dict = {}


def _register_ops():
    """Register the fused DVE op at runtime (visible to table-gen)."""
    import concourse.dve_ops as dve_ops
    from concourse.dve_spec import (
        C0,
        C1,
        Spec,
        Src0,
        Zero,
        _has_src1,
        lower,
        relu,
        select,
    )
    from concourse.dve_uop import DveOpSpec

    def make(name, body, ref):
        for op in dve_ops.OPS:
            if op.name == name:
                return op
        spec = Spec(body=body, reference=ref)
        row = 1 + len(dve_ops.OPS)
        assert row < 0x20
        shas = {}
        for ver in ("v3", "v4"):
            uops = lower(spec, ver=ver)
            shas[ver] = DveOpSpec(
                name=name, opcode=row, uops=uops, rd1_en=_has_src1(spec)
            ).sha(ver)
        op = dve_ops.DveOp(name, spec, subdim=False, uops_sha=shas)
        dve_ops._SUB_OPCODE_FOR_NAME[name] = row
        dve_ops.OPS.append(op)
        dve_ops.CUSTOM_DVE_SPECS[name] = spec
        return op

    # t = in0 + s0 ; out = (t < s1) ? max(t, 0) : 0
    def addsel_ref(in0, in1, s0, s1, imm2):
        t = (in0.astype(np.float32) + s0).astype(np.float32)
        return np.where(t < s1, np.maximum(t, 0.0), 0.0).astype(np.float32)

    t = Src0 + C0
    addsel = make("ADDSEL_ANT", select(t < C1, relu(t), Zero), addsel_ref)
    return addsel


def _build_program(Wks, UNION):
    import concourse.bacc as bacc
    import concourse.mybir as mybir
    import concourse.tile as tile

    addsel = _register_ops()

    WCOLS = BPC * P  # 768 weight columns
    Wmax = max(Wks)

    nc = bacc.Bacc("TRN2", target_bir_lowering=False, debug=False, num_devices=NCORES)
    f32 = mybir.dt.float32
    bf16 = mybir.dt.bfloat16

    # weights [:, :WCOLS] ++ moving [:, WCOLS:]
    wtmov = nc.dram_tensor("wtmov", [K, WCOLS + UNION], bf16, kind="ExternalInput")
    consts = nc.dram_tensor("consts", [P, 8], f32, kind="ExternalInput")
    out = nc.dram_tensor("out", [BPC * P, Wmax], bf16, kind="ExternalOutput")

    with tile.TileContext(nc) as tc:
        with (
            tc.tile_pool(name="const", bufs=1) as cpool,
            tc.tile_pool(name="work", bufs=4) as wpool,
            tc.tile_pool(name="outp", bufs=4) as spool,
            tc.tile_pool(name="psx", bufs=3, space="PSUM") as ppx,
        ):
            wm_t = cpool.tile([K, WCOLS + UNION], bf16, tag="wtmov")
            c_t = cpool.tile([P, 8], f32, tag="consts")
            warm_t = cpool.tile([P, 2], f32, tag="warm")

            # pull the ACT sqrt table in immediately (no DMA deps)
            nc.vector.memset(warm_t[:, 0:1], 1.0)
            nc.scalar.sqrt(warm_t[:, 1:2], warm_t[:, 0:1])

            # weights land first so LDWEIGHTS can issue early
            nc.sync.dma_start(wm_t[:, 0 : WCOLS + MMW], wtmov[:, 0 : WCOLS + MMW])
            nc.scalar.dma_start(c_t[:], consts[:])
            nc.sync.dma_start(wm_t[:, WCOLS + MMW :], wtmov[:, WCOLS + MMW :])

            for k in range(BPC):
                Wk = Wks[k]
                o = WCOLS + (UNION - Wk - P * (BPC - 1 - k))
                xb = ppx.tile([P, Wk], f32, tag="xb")
                # slots 0 and 5 are processed in 512-wide halves: slot 0 so the
                # DVE can start after the first matmul, slot 5 for a fast tail
                pieces = (
                    [(0, MMW), (MMW, Wk)] if k in (0, BPC - 1) else [(0, Wk)]
                )
                for h in range(0, Wk, MMW):
                    hw = min(MMW, Wk - h)
                    nc.tensor.matmul(
                        xb[:, h : h + hw],
                        wm_t[:, k * P : (k + 1) * P],
                        wm_t[:, o + h : o + h + hw],
                        start=True,
                        stop=True,
                    )
                for p0, p1 in pieces:
                    v = wpool.tile([P, p1 - p0], f32, tag="v")
                    nc.vector._custom_dve(
                        addsel,
                        out=v[:],
                        in0=xb[:, p0:p1],
                        s0=c_t[:, k : k + 1],
                        s1=c_t[:, 6:7],
                    )
                    s = spool.tile([P, p1 - p0], bf16, tag="s")
                    nc.scalar.sqrt(s[:], v[:])
                    nc.sync.dma_start(out[k * P : (k + 1) * P, p0:p1], s[:])

    nc.compile()
    return nc


def _get_program(Wks, UNION):
    with _lock:
        key = ("nc", Wks, UNION)
        if key not in _cache:
            _cache[key] = _build_program(Wks, UNION)
    return _cache[key]


def _split3_bf16_f64(v64: np.ndarray):
    """3-way bf16 split of float64 values: h+m+l captures ~24 mantissa bits."""
    import ml_dtypes

    bf = ml_dtypes.bfloat16
    h = v64.astype(bf)
    r1 = v64 - h.astype(np.float64)
    m = r1.astype(bf)
    r2 = r1 - m.astype(np.float64)
    l = r2.astype(bf)
    return h, m, l


def _prepare_inputs(species, coordinates, cutoff):
    import ml_dtypes

    bf = ml_dtypes.bfloat16
    coords = np.asarray(coordinates, dtype=np.float32).reshape(-1, 3).copy()
    assert coords.shape[0] == N
    valid = np.asarray(species).reshape(-1) >= 0
    if not valid.all():
        bad = np.where(~valid)[0]
        coords[bad] = (1.0e5 + 1.0e4 * np.arange(len(bad), dtype=np.float32))[:, None]

    cutf = np.float32(cutoff)
    perm = np.argsort(coords[:, 0], kind="stable")
    sx = np.ascontiguousarray(coords[perm, 0])
    sy = np.ascontiguousarray(coords[perm, 1])
    sz = np.ascontiguousarray(coords[perm, 2])
    svalid = valid[perm]

    # minimum per-slot window width so every block's cutoff neighborhood
    # is covered (slot k serves blocks {6c+k}; the SPMD program uses the
    # max over cores, rounded up; at least 544 so each slot has 2 pieces)
    bmins = sx[::P][:NB]
    j0 = np.searchsorted(sx, bmins - cutf, side="left")
    need = np.maximum(P * (np.arange(NB) + 1) - j0, 1)
    Wks = tuple(
        int(max(544, -(-int(need[k::BPC].max()) // 32) * 32)) for k in range(BPC)
    )
    assert max(Wks) <= 1024, f"window {max(Wks)} exceeds PSUM tile budget"
    UNION = max(Wks[k] + P * (BPC - 1 - k) for k in range(BPC))
    PADL = UNION - BPC * P
    WCOLS = BPC * P

    gx = np.concatenate([np.full(PADL, -1.0e4, np.float64), sx.astype(np.float64)])
    gy = np.concatenate([np.zeros(PADL, np.float64), sy.astype(np.float64)])
    gz = np.concatenate([np.zeros(PADL, np.float64), sz.astype(np.float64)])
    gvalid = np.concatenate([np.zeros(PADL, bool), svalid])

    cut2 = cutf * cutf
    cut_hi = np.nextafter(cut2, np.float32(np.inf), dtype=np.float32)

    def rows21(xc, yc, zc, r2):
        rows = []
        for d in (xc, yc, zc):
            h, m, l = _split3_bf16_f64(d)
            rows += [h, m, l, h, m, h]
        ra, rb, rc = _split3_bf16_f64(r2)
        rows += [ra, rb, rc]
        return rows

    in_maps = []
    for c in range(NCORES):
        base = PADL + WCOLS * (c + 1) - UNION
        rowi = np.arange(WCOLS * c, WCOLS * (c + 1))
        rx = sx[rowi].astype(np.float64)
        ry = sy[rowi].astype(np.float64)
        rz = sz[rowi].astype(np.float64)
        realr = np.abs(rx) <= 1000.0
        cx = float(np.median(rx[realr])) if realr.any() else 0.0
        cy = cz = 20.0

        # moving columns (union slice), centered
        ux = gx[base : base + UNION] - cx
        uy = gy[base : base + UNION] - cy
        uz = gz[base : base + UNION] - cz
        ur2 = ux * ux + uy * uy + uz * uz
        mov = rows21(ux, uy, uz, ur2)

        # stationary per-row weights
        wx, wy, wz = rx - cx, ry - cy, rz - cz
        wr2 = wx * wx + wy * wy + wz * wz
        wrows = []
        for d in (wx, wy, wz):
            h, m, l = _split3_bf16_f64(d)
            n2 = np.float32(-2.0)
            wrows += [
                (n2 * h.astype(np.float32)).astype(bf),
                (n2 * h.astype(np.float32)).astype(bf),
                (n2 * h.astype(np.float32)).astype(bf),
                (n2 * m.astype(np.float32)).astype(bf),
                (n2 * m.astype(np.float32)).astype(bf),
                (n2 * l.astype(np.float32)).astype(bf),
            ]
        ones = np.ones(WCOLS, bf)
        wrows += [ones, ones, ones]

        wtmov = np.empty((K, WCOLS + UNION), bf)
        for r in range(K):
            wtmov[r, :WCOLS] = wrows[r]
            wtmov[r, WCOLS:] = mov[r]

        consts = np.zeros((P, 8), np.float32)
        consts[:, 0:BPC] = wr2.astype(np.float32).reshape(BPC, P).T
        consts[:, 6] = cut_hi
        in_maps.append({"wtmov": wtmov, "consts": consts})

    _cache["meta"] = (perm, Wks, UNION, gvalid, PADL)
    return in_maps


def _run(in_maps, trace=False):
    from concourse import bass_utils

    meta = _cache["meta"]
    nc = _get_program(meta[1], meta[2])
    return bass_utils.run_bass_kernel_spmd(
        nc, in_maps, core_ids=list(range(NCORES)), trace=trace
    )


def _assemble(results, perm, Wks, gvalid, PADL):
    full = np.zeros((N, N), np.float32)
    svalid = gvalid[PADL:]
    for c in range(NCORES):
        slab = np.asarray(results[c]["out"]).astype(np.float32)
        for k in range(BPC):
            W = Wks[k]
            b = BPC * c + k
            start = P * (b + 1) - W
            t0 = max(0, -start)
            vals = slab[k * P : (k + 1) * P, t0:W]
            rr, cc = np.nonzero(vals)
            if rr.size == 0:
                continue
            si = P * b + rr
            sj = start + t0 + cc
            keep = svalid[si] & svalid[sj] & (si != sj)
            if not keep.all():
                rr, cc, si, sj = rr[keep], cc[keep], si[keep], sj[keep]
            oi = perm[si]
            oj = perm[sj]
            hi = np.maximum(oi, oj)
            lo = np.minimum(oi, oj)
            full[hi, lo] = vals[rr, cc]
    return full


def kernel(species, coordinates, cutoff):
    in_maps = _prepare_inputs(species, coordinates, cutoff)
    res = _run(in_maps)
    perm, Wks, UNION, gvalid, PADL = _cache["meta"]
    return _assemble(res.results, perm, Wks, gvalid, PADL)


# revision 12
# speedup vs baseline: 3.4583x; 1.1640x over previous
"""Pairwise distance screen (CellList) kernel for 8 Trainium2 NeuronCores.

Computes the masked dense [N, N] lower-triangular distance matrix:
  out[i, j] = sqrt(|c_i - c_j|^2)  if  j < i, both species valid, d2 <= cutoff^2
            = 0                    otherwise

Strategy (2D spatial bucketing + bilinear distance on the tensor engine):
  - Atoms are sorted into 8 x-strips of 768 (by x), y-sorted within each
    strip.  Core c owns strip c = 6 row-blocks of 128.  For each block b
    the host packs the exact candidate column set
      C_b = { j < 128(b+1) : x_j >= bxmin-cut, bymin-cut <= y_j <= bymax+cut }
    (block bounds over its real rows).  For any pair (i, j), i > j, within
    the cutoff, the block of i satisfies all three conditions for j, so
    every pair is covered; duplicates (diagonal-block pairs) scatter the
    same value twice, which is benign.
  - d2 is computed bilinearly on the PE: d2 = r2_j - 2*ci.cj + r2_i, with
    coordinates centered per slot (x, y at the block median, z at 20) and
    3-way bf16 split so every product is exact; the K=21 matmul
    accumulates -2*ci.cj + r2_j in fp32 PSUM (6 product terms per dim;
    r2_j is a 3-way split of the f64 value).  |d2 error| ~1e-4, so only
    O(1) pairs within ~1e-4 of the cutoff boundary can flip vs the
    reference mask (~5e-3 Frobenius budget vs the 2e-2 gate).
  - The packed column order is arbitrary: the host keeps a per-slot
    column map and scatters nonzero result entries to
    full[max(oi,oj), min(oi,oj)], dropping sentinel padding, dummy
    (species<0) atoms, and the diagonal.
  - DVE op ADDSEL adds r2_i ([P,1] per-partition) and selects
    relu(t) if t < nextafter(cut2) else 0 (relu guards sqrt(-eps) on the
    diagonal).  ACT computes sqrt -> bf16 output slabs.
  - Slots are emitted widest-first so the trailing slot drains quickly.
"""

import threading

import numpy as np

N = 6144
P = 128
NCORES = 8
BPC = 6  # row-blocks per core
NB = N // P  # 48
NSTRIP = 8
SS = N // NSTRIP  # 768 atoms per x-strip
MMW = 512  # matmul free-dim width (one PSUM bank)
K = 21  # 6 product terms per dim + 3 r2 rows

_lock = threading.Lock()
_cache: dict = {}


def _register_ops():
    """Register the fused DVE op at runtime (visible to table-gen)."""
    import concourse.dve_ops as dve_ops
    from concourse.dve_spec import (
        C0,
        C1,
        Spec,
        Src0,
        Zero,
        _has_src1,
        lower,
        relu,
        select,
    )
    from concourse.dve_uop import DveOpSpec

    def make(name, body, ref):
        for op in dve_ops.OPS:
            if op.name == name:
                return op
        spec = Spec(body=body, reference=ref)
        row = 1 + len(dve_ops.OPS)
        assert row < 0x20
        shas = {}
        for ver in ("v3", "v4"):
            uops = lower(spec, ver=ver)
            shas[ver] = DveOpSpec(
                name=name, opcode=row, uops=uops, rd1_en=_has_src1(spec)
            ).sha(ver)
        op = dve_ops.DveOp(name, spec, subdim=False, uops_sha=shas)
        dve_ops._SUB_OPCODE_FOR_NAME[name] = row
        dve_ops.OPS.append(op)
        dve_ops.CUSTOM_DVE_SPECS[name] = spec
        return op

    # t = in0 + s0 ; out = (t < s1) ? max(t, 0) : 0
    def addsel_ref(in0, in1, s0, s1, imm2):
        t = (in0.astype(np.float32) + s0).astype(np.float32)
        return np.where(t < s1, np.maximum(t, 0.0), 0.0).astype(np.float32)

    t = Src0 + C0
    addsel = make("ADDSEL_ANT", select(t < C1, relu(t), Zero), addsel_ref)
    return addsel


def _build_program(Wks):
    import concourse.bacc as bacc
    import concourse.mybir as mybir
    import concourse.tile as tile

    addsel = _register_ops()

    WCOLS = BPC * P  # 768 weight columns
    MTOT = sum(Wks)
    Wmax = max(Wks)
    offs = [sum(Wks[:k]) for k in range(BPC)]
    order = sorted(range(BPC), key=lambda k: -Wks[k])  # widest first

    nc = bacc.Bacc("TRN2", target_bir_lowering=False, debug=False, num_devices=NCORES)
    f32 = mybir.dt.float32
    bf16 = mybir.dt.bfloat16

    # weights [:, :WCOLS] ++ packed moving columns [:, WCOLS:]
    wtmov = nc.dram_tensor("wtmov", [K, WCOLS + MTOT], bf16, kind="ExternalInput")
    consts = nc.dram_tensor("consts", [P, 8], f32, kind="ExternalInput")
    out = nc.dram_tensor("out", [BPC * P, Wmax], bf16, kind="ExternalOutput")

    k0 = order[0]
    head = WCOLS + offs[k0] + Wks[k0]  # weights + first slot's columns

    with tile.TileContext(nc) as tc:
        with (
            tc.tile_pool(name="const", bufs=1) as cpool,
            tc.tile_pool(name="work", bufs=4) as wpool,
            tc.tile_pool(name="outp", bufs=4) as spool,
            tc.tile_pool(name="psx", bufs=3, space="PSUM") as ppx,
        ):
            wm_t = cpool.tile([K, WCOLS + MTOT], bf16, tag="wtmov")
            c_t = cpool.tile([P, 8], f32, tag="consts")
            warm_t = cpool.tile([P, 2], f32, tag="warm")

            # pull the ACT sqrt table in immediately (no DMA deps)
            nc.vector.memset(warm_t[:, 0:1], 1.0)
            nc.scalar.sqrt(warm_t[:, 1:2], warm_t[:, 0:1])

            if head < WCOLS + MTOT:
                nc.sync.dma_start(wm_t[:, 0:head], wtmov[:, 0:head])
                nc.sync.dma_start(wm_t[:, head:], wtmov[:, head:])
            else:
                nc.sync.dma_start(wm_t[:], wtmov[:])
            nc.scalar.dma_start(c_t[:], consts[:])

            for k in order:
                Wk = Wks[k]
                o = WCOLS + offs[k]
                xb = ppx.tile([P, Wk], f32, tag="xb")
                for h in range(0, Wk, MMW):
                    hw = min(MMW, Wk - h)
                    nc.tensor.matmul(
                        xb[:, h : h + hw],
                        wm_t[:, k * P : (k + 1) * P],
                        wm_t[:, o + h : o + h + hw],
                        start=True,
                        stop=True,
                    )
                v = wpool.tile([P, Wk], f32, tag="v")
                nc.vector._custom_dve(
                    addsel,
                    out=v[:],
                    in0=xb[:],
                    s0=c_t[:, k : k + 1],
                    s1=c_t[:, 6:7],
                )
                s = spool.tile([P, Wk], bf16, tag="s")
                nc.scalar.sqrt(s[:], v[:])
                nc.sync.dma_start(out[k * P : (k + 1) * P, 0:Wk], s[:])

    nc.compile()
    return nc


def _get_program(Wks):
    with _lock:
        key = ("nc", Wks)
        if key not in _cache:
            _cache[key] = _build_program(Wks)
    return _cache[key]


def _split3_bf16_f64(v64: np.ndarray):
    """3-way bf16 split of float64 values: h+m+l captures ~24 mantissa bits."""
    import ml_dtypes

    bf = ml_dtypes.bfloat16
    h = v64.astype(bf)
    r1 = v64 - h.astype(np.float64)
    m = r1.astype(bf)
    r2 = r1 - m.astype(np.float64)
    l = r2.astype(bf)
    return h, m, l


def _mov21(xc, yc, zc, r2):
    rows = []
    for d in (xc, yc, zc):
        h, m, l = _split3_bf16_f64(d)
        rows += [h, m, l, h, m, h]
    ra, rb, rc = _split3_bf16_f64(r2)
    rows += [ra, rb, rc]
    return rows


def _prepare_inputs(species, coordinates, cutoff):
    import ml_dtypes

    bf = ml_dtypes.bfloat16
    coords = np.asarray(coordinates, dtype=np.float32).reshape(-1, 3).copy()
    assert coords.shape[0] == N
    valid = np.asarray(species).reshape(-1) >= 0
    if not valid.all():
        bad = np.where(~valid)[0]
        coords[bad] = (1.0e5 + 1.0e4 * np.arange(len(bad), dtype=np.float32))[:, None]

    cutf = float(np.float32(cutoff))
    x, y = coords[:, 0], coords[:, 1]
    p1 = np.argsort(x, kind="stable")
    perm = np.concatenate(
        [
            p1[s * SS : (s + 1) * SS][
                np.argsort(y[p1[s * SS : (s + 1) * SS]], kind="stable")
            ]
            for s in range(NSTRIP)
        ]
    )
    sx = coords[perm, 0].astype(np.float64)
    sy = coords[perm, 1].astype(np.float64)
    sz = coords[perm, 2].astype(np.float64)
    svalid = valid[perm]

    # exact candidate column sets per block
    cols = []
    sizes = np.zeros(NB, np.int64)
    for b in range(NB):
        rows = slice(P * b, P * b + P)
        rv = svalid[rows]
        if not rv.any():
            cols.append(np.empty(0, np.int64))
            continue
        rxv = sx[rows][rv]
        ryv = sy[rows][rv]
        bxmin = rxv.min()
        bymin, bymax = ryv.min(), ryv.max()
        j = np.arange(P * (b + 1))
        m = (sx[j] >= bxmin - cutf) & (sy[j] >= bymin - cutf) & (sy[j] <= bymax + cutf)
        cb = j[m]
        cols.append(cb)
        sizes[b] = len(cb)

    Wks = tuple(
        int(max(64, -(-int(sizes[k::BPC].max()) // 32) * 32)) for k in range(BPC)
    )
    assert max(Wks) <= 1024, f"slot width {max(Wks)} exceeds PSUM tile budget"
    MTOT = sum(Wks)
    WCOLS = BPC * P

    cut2 = np.float32(cutf) * np.float32(cutf)
    cut_hi = np.nextafter(cut2, np.float32(np.inf), dtype=np.float32)

    in_maps = []
    colmaps = []
    for c in range(NCORES):
        wtmov = np.empty((K, WCOLS + MTOT), bf)
        consts = np.zeros((P, 8), np.float32)
        consts[:, 6] = cut_hi
        cmaps = []
        off = WCOLS
        for k in range(BPC):
            b = BPC * c + k
            Wk = Wks[k]
            rows = slice(P * b, P * b + P)
            rv = svalid[rows]
            rx, ry, rz = sx[rows], sy[rows], sz[rows]
            cx = float(np.median(rx[rv])) if rv.any() else 0.0
            cy = float(np.median(ry[rv])) if rv.any() else 0.0
            cz = 20.0

            cb = cols[b]
            ux = np.full(Wk, -1.0e4, np.float64)
            uy = np.zeros(Wk, np.float64)
            uz = np.zeros(Wk, np.float64)
            ux[: len(cb)] = sx[cb]
            uy[: len(cb)] = sy[cb]
            uz[: len(cb)] = sz[cb]
            ux -= cx
            uy -= cy
            uz -= cz
            for r, row in enumerate(_mov21(ux, uy, uz, ux * ux + uy * uy + uz * uz)):
                wtmov[r, off : off + Wk] = row

            wx, wy, wz = rx - cx, ry - cy, rz - cz
            wr2 = wx * wx + wy * wy + wz * wz
            wrows = []
            for d in (wx, wy, wz):
                h, m, l = _split3_bf16_f64(d)
                n2 = np.float32(-2.0)
                h2 = (n2 * h.astype(np.float32)).astype(bf)
                m2 = (n2 * m.astype(np.float32)).astype(bf)
                l2 = (n2 * l.astype(np.float32)).astype(bf)
                wrows += [h2, h2, h2, m2, m2, l2]
            ones = np.ones(P, bf)
            wrows += [ones, ones, ones]
            for r in range(K):
                wtmov[r, k * P : (k + 1) * P] = wrows[r]
            consts[:, k] = wr2.astype(np.float32)

            cmap = np.full(Wk, -1, np.int64)
            cmap[: len(cb)] = cb
            cmaps.append(cmap)
            off += Wk
        in_maps.append({"wtmov": wtmov, "consts": consts})
        colmaps.append(cmaps)

    _cache["meta"] = (perm, Wks, svalid, colmaps)
    return in_maps


def _run(in_maps, trace=False):
    from concourse import bass_utils

    nc = _get_program(_cache["meta"][1])
    return bass_utils.run_bass_kernel_spmd(
        nc, in_maps, core_ids=list(range(NCORES)), trace=trace
    )


def _assemble(results, perm, Wks, svalid, colmaps):
    full = np.zeros((N, N), np.float32)
    for c in range(NCORES):
        slab = np.asarray(results[c]["out"]).astype(np.float32)
        for k in range(BPC):
            b = BPC * c + k
            vals = slab[k * P : (k + 1) * P, 0 : Wks[k]]
            rr, cc = np.nonzero(vals)
            if rr.size == 0:
                continue
            si = P * b + rr
            sj = colmaps[c][k][cc]
            keep = (sj >= 0) & svalid[si] & (si != sj)
            keep &= svalid[np.maximum(sj, 0)]
            if not keep.all():
                rr, cc, si, sj = rr[keep], cc[keep], si[keep], sj[keep]
            oi = perm[si]
            oj = perm[sj]
            hi = np.maximum(oi, oj)
            lo = np.minimum(oi, oj)
            full[hi, lo] = vals[rr, cc]
    return full


def kernel(species, coordinates, cutoff):
    in_maps = _prepare_inputs(species, coordinates, cutoff)
    res = _run(in_maps)
    perm, Wks, svalid, colmaps = _cache["meta"]
    return _assemble(res.results, perm, Wks, svalid, colmaps)
